# revision 16
# baseline (speedup 1.0000x reference)
"""Multi-head attention block (B=2, N=2048, C=1024, H=16) on 8 TRN2 NeuronCores.

Sharding: tensor-parallel over heads. Core c owns global heads {2c, 2c+1}:
  - w_qkv columns for q/k/v of those heads  -> [1024, 384] slice
  - w_proj rows for those heads             -> [128, 1024] slice
  - x is replicated, pre-transposed on host to xT [1024, 4096]
Each core computes a full [4096, 1024] partial projection output; the host
sums the 8 partials and adds b_proj.

Device pipeline per core (bf16 matmuls, fp32 PSUM):
  1. qkvT = w_slice.T @ xT  -> qT/kT/vT in [head_dim, seq] layout
  2. attention processed per (batch, 512-wide q chunk): both heads' scores^T
     [keys=128, 512] packed into one [128, 1024] PSUM tile via row-group
     tile_position (concurrent on the PE), one Exp per chunk (scale folded),
     V-matmul with full-128-column stacked weights [v | ones | pad | dup]
     (keeps the PE array fully active + FWL) accumulating out^T and the
     softmax denominators; normalize via fp32 reciprocal + partition_broadcast.
  3. out^T chunks feed the projection matmul as lhsT; result DMA'd out f32.
qkv for batch 1 and the projection matmuls are interleaved into the attention
loop (background queue) to fill PE slack under the ScalarE exp stream.
"""

import math
import numpy as np

import concourse.mybir as mybir
import concourse.tile as tile
from concourse import bacc
from concourse.bass_utils import run_bass_kernel_spmd
from concourse.masks import make_identity

F32 = mybir.dt.float32
MMDT = mybir.dt.bfloat16  # matmul operand dtype

# Problem shape (hardcoded per contract)
B, N, C, H = 2, 2048, 1024, 16
D = C // H            # 64 head dim
SEQ = B * N           # 4096
NCORES = 8
HL = H // NCORES      # 2 local heads per core
MW = 3 * HL * D       # 384 w_qkv slice cols (q|k|v for 2 heads)
KT = C // 128         # 8 contraction tiles for the projections
SC = 512              # seq chunk for qkv stage
NSC = SEQ // SC       # 8
KCN = N // 128        # 16 key chunks per batch
QW = 512              # q-chunk width for attention
NQH = N // QW         # 4
SCALE = 1.0 / math.sqrt(D)


def build_nc():
    nc = bacc.Bacc("TRN2", target_bir_lowering=False, debug=False)
    xt_d = nc.dram_tensor("xt", [C, SEQ], MMDT, kind="ExternalInput")
    wqkv_d = nc.dram_tensor("wqkv", [C, MW], MMDT, kind="ExternalInput")
    wproj_d = nc.dram_tensor("wproj", [HL * D, C], MMDT, kind="ExternalInput")
    out_d = nc.dram_tensor("out", [SEQ, C], F32, kind="ExternalOutput")

    with tile.TileContext(nc) as tc:
        with (
            tc.tile_pool(name="const", bufs=1) as const,
            tc.tile_pool(name="xin", bufs=2) as xin,
            tc.tile_pool(name="qkvt", bufs=1) as qkvt,
            tc.tile_pool(name="vaugp", bufs=2) as vaugp,
            tc.tile_pool(name="ptp", bufs=3) as ptp,
            tc.tile_pool(name="outt", bufs=2) as outtp,
            tc.tile_pool(name="rp", bufs=2) as rp,
            tc.tile_pool(name="op", bufs=3) as op,
            tc.tile_pool(name="ps_st", bufs=2, space="PSUM") as ps_st,
            tc.tile_pool(name="ps_v", bufs=1, space="PSUM") as ps_v,
            tc.tile_pool(name="ps_aux", bufs=2, space="PSUM") as ps_aux,
        ):
            # ---- constants ----
            ident = const.tile([128, 128], MMDT, tag="ident")
            ones_sb = const.tile([128, 1], F32, tag="ones")
            w_sb = const.tile([128, KT, MW], MMDT, tag="wqkv")
            wp_sb = const.tile([128, C], MMDT, tag="wproj")
            # first data the pipeline needs, in order
            for kt in range(KT):
                nc.sync.dma_start(
                    w_sb[:, kt, :], wqkv_d.ap()[kt * 128 : (kt + 1) * 128, :]
                )
            make_identity(nc, ident[:])
            nc.gpsimd.memset(ones_sb[:], 1.0)
            nc.sync.dma_start(wp_sb[:], wproj_d.ap())

            # persistent transposed qkv: [dim-of-2-heads=128, seq]
            q_sb = qkvt.tile([128, SEQ], MMDT, tag="q")
            k_sb = qkvt.tile([128, SEQ], MMDT, tag="k")
            v_sb = qkvt.tile([128, SEQ], MMDT, tag="v")
            dst = [q_sb, k_sb, v_sb]

            # full xT resident; strip DMAs issued upfront, consumed as they land
            xt_all = qkvt.tile([128, KT, SEQ], MMDT, tag="xt_all")
            for sc in range(NSC):
                for kt in range(KT):
                    nc.sync.dma_start(
                        xt_all[:, kt, sc * SC : (sc + 1) * SC],
                        xt_d.ap()[kt * 128 : (kt + 1) * 128, sc * SC : (sc + 1) * SC],
                    )

            # ---- background queue with markers ----
            bg = []  # (key|None, closure)
            done = set()

            def pump(n=1):
                for _ in range(n):
                    if not bg:
                        return
                    key, fn = bg.pop(0)
                    fn()
                    if key is not None:
                        done.add(key)

            def pump_until(key):
                while key not in done:
                    k, fn = bg.pop(0)
                    fn()
                    if k is not None:
                        done.add(k)

            # ---- qkv + vaug emission (all via bg) ----
            def emit_qkv_part(holder, sc, m, part, nparts=4):
                if part == 0:
                    holder["ps"] = ps_aux.tile([128, SC], F32, tag="aux", name="qkv_ps")
                ps = holder["ps"]
                step = KT // nparts
                for kt in range(part * step, (part + 1) * step):
                    nc.tensor.matmul(
                        ps[:],
                        w_sb[:, kt, m * 128 : (m + 1) * 128],
                        xt_all[:, kt, sc * SC : (sc + 1) * SC],
                        start=(kt == 0),
                        stop=(kt == KT - 1),
                    )
                if part == nparts - 1:
                    nc.vector.tensor_copy(
                        out=dst[m][:, sc * SC : (sc + 1) * SC], in_=ps[:]
                    )

            vaug_store = {}

            def emit_vaug_tr(holder, b, h, piece):
                b0 = b * N
                if piece == 0:
                    holder.setdefault("tr", {})
                tr = ps_aux.tile([128, 4 * D], MMDT, tag="aux", name="tr")
                for tt in range(4):
                    t = piece * 4 + tt
                    nc.tensor.transpose(
                        tr[:, tt * D : (tt + 1) * D],
                        v_sb[h * D : (h + 1) * D, b0 + t * 128 : b0 + (t + 1) * 128],
                        ident[h * D : (h + 1) * D, h * D : (h + 1) * D],
                    )
                va = vaugp.tile([128, 4, 128], MMDT, tag=f"vaug{h}_{piece}", name="va")
                tr3 = tr[:].rearrange("p (a d) -> p a d", a=4)
                nc.vector.tensor_copy(out=va[:, :, :D], in_=tr3)
                nc.vector.tensor_copy(
                    out=va[:, :, D : D + 2],
                    in_=ones_sb[:, None, :].to_broadcast([128, 4, 2]),
                )
                nc.vector.tensor_copy(out=va[:, :, D + 2 :], in_=tr3[:, :, : 128 - D - 2])
                vaug_store[(b, h, piece)] = va

            # queue per batch: per sc: dma, k, v, vaug pieces, q
            for b in range(B):
                for scl in range(NSC // 2):
                    sc = b * (NSC // 2) + scl
                    holder = {}
                    for m in (1, 2, 0):  # k, v, then q
                        for part in range(4):
                            key = None
                            if part == 3:
                                key = ("kvq"[0 if m == 1 else 1 if m == 2 else 2], b, scl)
                            bg.append(
                                (
                                    key,
                                    lambda sc=sc, m=m, part=part, holder=holder: (
                                        emit_qkv_part(holder, sc, m, part)
                                    ),
                                )
                            )
                    for h in range(HL):
                        bg.append(
                            (
                                ("vaug", b, h, scl),
                                lambda b=b, h=h, scl=scl: emit_vaug_tr({}, b, h, scl),
                            )
                        )

            def emit_proj_chunk(outt, b0, s2, nck, use_act=False):
                pp = ps_aux.tile([128, 512], F32, tag="aux", name="proj_ps")
                nc.tensor.matmul(
                    pp[:],
                    outt[:, s2 * 128 : (s2 + 1) * 128],
                    wp_sb[:, nck * 512 : (nck + 1) * 512],
                    start=True,
                    stop=True,
                )
                o_sb = op.tile([128, 512], F32, tag="o", name="o_sb")
                if use_act:
                    nc.scalar.copy(out=o_sb[:], in_=pp[:])
                else:
                    nc.vector.tensor_copy(out=o_sb[:], in_=pp[:])
                nc.sync.dma_start(
                    out_d.ap()[
                        b0 + s2 * 128 : b0 + (s2 + 1) * 128,
                        nck * 512 : (nck + 1) * 512,
                    ],
                    o_sb[:],
                )

            # ---- attention ----
            for b in range(B):
                b0 = b * N
                outt = outtp.tile([128, N], MMDT, tag="outT")
                outu = [
                    outtp.tile([D, N], MMDT, tag=f"outu{h}", name="outu")
                    for h in range(HL)
                ]
                rs = [rp.tile([1, N], F32, tag=f"r{h}", name="rs") for h in range(HL)]

                for qh in range(NQH):
                    q0 = b0 + qh * QW
                    pump_until(("q", b, qh))
                    vps = [
                        ps_v.tile([128, QW], F32, tag=f"vps{h}", name=f"vps{h}")
                        for h in range(HL)
                    ]

                    def emit_scores(kc, q0=q0, b0=b0, b=b):
                        kcr = kc // 4
                        pump_until(("v", b, kcr))
                        for h in range(HL):
                            pump_until(("vaug", b, h, kcr))
                        st = ps_st.tile([128, 2 * QW], F32, tag="st", name="st")
                        for h in range(HL):
                            hs = slice(h * D, (h + 1) * D)
                            nc.tensor.matmul(
                                st[:, h * QW : (h + 1) * QW],
                                k_sb[hs, b0 + kc * 128 : b0 + (kc + 1) * 128],
                                q_sb[hs, q0 : q0 + QW],
                                start=True,
                                stop=True,
                                tile_position=(h * D, 0),
                            )
                        return st

                    pend = emit_scores(0)
                    for kc in range(KCN):
                        nxt = emit_scores(kc + 1) if kc + 1 < KCN else None
                        if kc < KCN - 2:
                            pump(1 if b == 0 else 2)
                        pt = ptp.tile([128, 2 * QW], MMDT, tag="pt")
                        nc.scalar.activation(
                            out=pt[:],
                            in_=pend[:],
                            func=mybir.ActivationFunctionType.Exp,
                            scale=SCALE,
                        )
                        for h in range(HL):
                            nc.tensor.matmul(
                                vps[h][:],
                                vaug_store[(b, h, kc // 4)][:, kc % 4, :],
                                pt[:, h * QW : (h + 1) * QW],
                                start=(kc == 0),
                                stop=(kc == KCN - 1),
                            )
                        pend = nxt

                    # drain vps to SBUF fast (frees PSUM for the next chunk)
                    qs = slice(qh * QW, (qh + 1) * QW)
                    for h in range(HL):
                        nc.vector.tensor_copy(out=outu[h][:, qs], in_=vps[h][:D, :])
                        nc.vector.tensor_copy(
                            out=rs[h][:, qs], in_=vps[h][D : D + 1, :]
                        )

                    # normalize off the critical path; queue this q-range's
                    # projection chunks as background work
                    def norm_and_proj(b0=b0, qh=qh, outt=outt, outu=outu, rs=rs, b=b):
                        for h in range(HL):
                            qs = slice(qh * QW, (qh + 1) * QW)
                            rb = rp.tile([D, QW], F32, tag="rb", name="rb")
                            nc.gpsimd.partition_broadcast(rb[:], rs[h][:, qs])
                            rbr = rp.tile([D, QW], F32, tag="rbr", name="rbr")
                            nc.vector.reciprocal_approx_fast(out=rbr[:], in_=rb[:])
                            nc.vector.tensor_mul(
                                out=outt[h * D : (h + 1) * D, qs],
                                in0=outu[h][:, qs],
                                in1=rbr[:],
                            )
                        tail = b == B - 1 and qh == NQH - 1
                        for s2 in range(qh * (QW // 128), (qh + 1) * (QW // 128)):
                            for nck in range(C // 512):
                                ua = tail and (s2 + nck) % 2 == 1
                                bg.append(
                                    (
                                        None,
                                        lambda outt=outt, b0=b0, s2=s2, nck=nck, ua=ua: (
                                            emit_proj_chunk(outt, b0, s2, nck, ua)
                                        ),
                                    )
                                )

                    bg.append((None, norm_and_proj))

            # drain remaining background work
            while bg:
                pump(1)
    nc.compile()
    return nc


_NC_CACHE = {}


def _get_nc():
    if "nc" not in _NC_CACHE:
        _NC_CACHE["nc"] = build_nc()
    return _NC_CACHE["nc"]


def make_in_maps(x, w_qkv, w_proj):
    np_dt = mybir.dt.np(MMDT)
    x = np.asarray(x, dtype=np.float32)
    w_qkv = np.asarray(w_qkv, dtype=np.float32)
    w_proj = np.asarray(w_proj, dtype=np.float32)
    xt = np.ascontiguousarray(x.reshape(SEQ, C).T.astype(np_dt))
    in_maps = []
    for c in range(NCORES):
        cs = slice(128 * c, 128 * c + 128)
        wslice = np.ascontiguousarray(
            np.concatenate(
                [w_qkv[:, cs], w_qkv[:, C:][:, cs], w_qkv[:, 2 * C :][:, cs]], axis=1
            ).astype(np_dt)
        )
        in_maps.append(
            {
                "xt": xt,
                "wqkv": wslice,
                "wproj": np.ascontiguousarray(w_proj[cs, :].astype(np_dt)),
            }
        )
    return in_maps


def kernel(x, w_qkv, w_proj, b_proj, _run_kwargs=None):
    nc = _get_nc()
    in_maps = make_in_maps(x, w_qkv, w_proj)
    res = run_bass_kernel_spmd(
        nc, in_maps, core_ids=list(range(NCORES)), **(_run_kwargs or {})
    )
    acc = res.results[0]["out"].astype(np.float32)
    for c in range(1, NCORES):
        acc = acc + res.results[c]["out"]
    acc = acc + np.asarray(b_proj, dtype=np.float32)[None, :]
    out = acc.reshape(B, N, C)
    if _run_kwargs:
        kernel.last_result = res
    return out


# revision 17
# speedup vs baseline: 1.0480x; 1.0480x over previous
"""Multi-head attention block (B=2, N=2048, C=1024, H=16) on 8 TRN2 NeuronCores.

Sharding: tensor-parallel over heads. Core c owns global heads {2c, 2c+1}:
  - w_qkv columns for q/k/v of those heads  -> [1024, 384] slice
  - w_proj rows for those heads             -> [128, 1024] slice
  - x is replicated, pre-transposed on host to xT [1024, 4096]
Each core computes a full [4096, 1024] partial projection output; the host
sums the 8 partials and adds b_proj.

Device pipeline per core (bf16 matmuls, fp32 PSUM):
  1. qkvT = w_slice.T @ xT  -> qT/kT/vT in [head_dim, seq] layout
  2. attention processed per (batch, 512-wide q chunk): both heads' scores^T
     [keys=128, 512] packed into one [128, 1024] PSUM tile via row-group
     tile_position (concurrent on the PE), one Exp per chunk (scale folded),
     V-matmul with full-128-column stacked weights [v | ones | pad | dup]
     (keeps the PE array fully active + FWL) accumulating out^T and the
     softmax denominators; normalize via fp32 reciprocal + partition_broadcast.
  3. out^T chunks feed the projection matmul as lhsT; result DMA'd out f32.
qkv for batch 1 and the projection matmuls are interleaved into the attention
loop (background queue) to fill PE slack under the ScalarE exp stream.
"""

import math
import numpy as np

import concourse.mybir as mybir
import concourse.tile as tile
from concourse import bacc
from concourse.bass_utils import run_bass_kernel_spmd
from concourse.masks import make_identity

F32 = mybir.dt.float32
MMDT = mybir.dt.bfloat16  # matmul operand dtype

# Problem shape (hardcoded per contract)
B, N, C, H = 2, 2048, 1024, 16
D = C // H            # 64 head dim
SEQ = B * N           # 4096
NCORES = 8
HL = H // NCORES      # 2 local heads per core
MW = 3 * HL * D       # 384 w_qkv slice cols (q|k|v for 2 heads)
KT = C // 128         # 8 contraction tiles for the projections
SC = 512              # seq chunk for qkv stage
NSC = SEQ // SC       # 8
KCN = N // 128        # 16 key chunks per batch
QW = 512              # q-chunk width for attention
NQH = N // QW         # 4
SCALE = 1.0 / math.sqrt(D)


def build_nc():
    nc = bacc.Bacc("TRN2", target_bir_lowering=False, debug=False)
    xt_d = nc.dram_tensor("xt", [C, SEQ], MMDT, kind="ExternalInput")
    wqkv_d = nc.dram_tensor("wqkv", [C, MW], MMDT, kind="ExternalInput")
    wproj_d = nc.dram_tensor("wproj", [HL * D, C], MMDT, kind="ExternalInput")
    out_d = nc.dram_tensor("out", [SEQ, C], F32, kind="ExternalOutput")

    with tile.TileContext(nc) as tc:
        with (
            tc.tile_pool(name="const", bufs=1) as const,
            tc.tile_pool(name="xin", bufs=2) as xin,
            tc.tile_pool(name="qkvt", bufs=1) as qkvt,
            tc.tile_pool(name="vaugp", bufs=2) as vaugp,
            tc.tile_pool(name="ptp", bufs=3) as ptp,
            tc.tile_pool(name="outt", bufs=2) as outtp,
            tc.tile_pool(name="rp", bufs=2) as rp,
            tc.tile_pool(name="op", bufs=3) as op,
            tc.tile_pool(name="ps_st", bufs=2, space="PSUM") as ps_st,
            tc.tile_pool(name="ps_v", bufs=1, space="PSUM") as ps_v,
            tc.tile_pool(name="ps_aux", bufs=2, space="PSUM") as ps_aux,
        ):
            # ---- constants ----
            ident = const.tile([128, 128], MMDT, tag="ident")
            ones_sb = const.tile([128, 1], F32, tag="ones")
            w_sb = const.tile([128, KT, MW], MMDT, tag="wqkv")
            wp_sb = const.tile([128, C], MMDT, tag="wproj")
            # first data the pipeline needs, in order
            for kt in range(KT):
                nc.sync.dma_start(
                    w_sb[:, kt, :], wqkv_d.ap()[kt * 128 : (kt + 1) * 128, :]
                )
            make_identity(nc, ident[:])
            nc.gpsimd.memset(ones_sb[:], 1.0)
            nc.sync.dma_start(wp_sb[:], wproj_d.ap())

            # persistent transposed qkv: [dim-of-2-heads=128, seq]
            q_sb = qkvt.tile([128, SEQ], MMDT, tag="q")
            k_sb = qkvt.tile([128, SEQ], MMDT, tag="k")
            v_sb = qkvt.tile([128, SEQ], MMDT, tag="v")
            dst = [q_sb, k_sb, v_sb]

            # full xT resident; strip DMAs issued upfront, consumed as they land
            xt_all = qkvt.tile([128, KT, SEQ], MMDT, tag="xt_all")
            for sc in range(NSC):
                for kt in range(KT):
                    nc.sync.dma_start(
                        xt_all[:, kt, sc * SC : (sc + 1) * SC],
                        xt_d.ap()[kt * 128 : (kt + 1) * 128, sc * SC : (sc + 1) * SC],
                    )

            # ---- background queue with markers ----
            bg = []  # (key|None, closure)
            done = set()

            def pump(n=1):
                for _ in range(n):
                    if not bg:
                        return
                    key, fn = bg.pop(0)
                    fn()
                    if key is not None:
                        done.add(key)

            def pump_until(key):
                while key not in done:
                    k, fn = bg.pop(0)
                    fn()
                    if k is not None:
                        done.add(k)

            # ---- qkv + vaug emission (all via bg) ----
            def emit_qkv_part(holder, sc, m, part, nparts=4):
                if part == 0:
                    holder["ps"] = ps_aux.tile([128, SC], F32, tag="aux", name="qkv_ps")
                ps = holder["ps"]
                step = KT // nparts
                for kt in range(part * step, (part + 1) * step):
                    nc.tensor.matmul(
                        ps[:],
                        w_sb[:, kt, m * 128 : (m + 1) * 128],
                        xt_all[:, kt, sc * SC : (sc + 1) * SC],
                        start=(kt == 0),
                        stop=(kt == KT - 1),
                    )
                if part == nparts - 1:
                    nc.vector.tensor_copy(
                        out=dst[m][:, sc * SC : (sc + 1) * SC], in_=ps[:]
                    )

            vaug_store = {}

            def emit_vaug_tr(holder, b, h, piece):
                b0 = b * N
                if piece == 0:
                    holder.setdefault("tr", {})
                tr = ps_aux.tile([128, 4 * D], MMDT, tag="aux", name="tr")
                for tt in range(4):
                    t = piece * 4 + tt
                    nc.tensor.transpose(
                        tr[:, tt * D : (tt + 1) * D],
                        v_sb[h * D : (h + 1) * D, b0 + t * 128 : b0 + (t + 1) * 128],
                        ident[h * D : (h + 1) * D, h * D : (h + 1) * D],
                    )
                va = vaugp.tile([128, 4, 128], MMDT, tag=f"vaug{h}_{piece}", name="va")
                tr3 = tr[:].rearrange("p (a d) -> p a d", a=4)
                nc.vector.tensor_copy(out=va[:, :, :D], in_=tr3)
                nc.vector.tensor_copy(
                    out=va[:, :, D : D + 2],
                    in_=ones_sb[:, None, :].to_broadcast([128, 4, 2]),
                )
                nc.vector.tensor_copy(out=va[:, :, D + 2 :], in_=tr3[:, :, : 128 - D - 2])
                vaug_store[(b, h, piece)] = va

            # queue per batch: per sc: dma, k, v, vaug pieces, q
            for b in range(B):
                for scl in range(NSC // 2):
                    sc = b * (NSC // 2) + scl
                    holder = {}
                    for m in (1, 2, 0):  # k, v, then q
                        for part in range(4):
                            key = None
                            if part == 3:
                                key = ("kvq"[0 if m == 1 else 1 if m == 2 else 2], b, scl)
                            bg.append(
                                (
                                    key,
                                    lambda sc=sc, m=m, part=part, holder=holder: (
                                        emit_qkv_part(holder, sc, m, part)
                                    ),
                                )
                            )
                    for h in range(HL):
                        bg.append(
                            (
                                ("vaug", b, h, scl),
                                lambda b=b, h=h, scl=scl: emit_vaug_tr({}, b, h, scl),
                            )
                        )

            def emit_proj_chunk(outt, b0, s2, nck, use_act=False):
                if use_act:
                    pp = ps_v.tile([128, 512], F32, tag="vps0", name="proj_ps")
                else:
                    pp = ps_aux.tile([128, 512], F32, tag="aux", name="proj_ps")
                nc.tensor.matmul(
                    pp[:],
                    outt[:, s2 * 128 : (s2 + 1) * 128],
                    wp_sb[:, nck * 512 : (nck + 1) * 512],
                    start=True,
                    stop=True,
                )
                o_sb = op.tile([128, 512], F32, tag="o", name="o_sb")
                if use_act:
                    nc.scalar.copy(out=o_sb[:], in_=pp[:])
                else:
                    nc.vector.tensor_copy(out=o_sb[:], in_=pp[:])
                nc.sync.dma_start(
                    out_d.ap()[
                        b0 + s2 * 128 : b0 + (s2 + 1) * 128,
                        nck * 512 : (nck + 1) * 512,
                    ],
                    o_sb[:],
                )

            # ---- attention ----
            for b in range(B):
                b0 = b * N
                outt = outtp.tile([128, N], MMDT, tag="outT")
                outu = [
                    outtp.tile([D, N], MMDT, tag=f"outu{h}", name="outu")
                    for h in range(HL)
                ]
                rs = [rp.tile([1, N], F32, tag=f"r{h}", name="rs") for h in range(HL)]

                for qh in range(NQH):
                    q0 = b0 + qh * QW
                    pump_until(("q", b, qh))
                    vps = [
                        ps_v.tile([128, QW], F32, tag=f"vps{h}", name=f"vps{h}")
                        for h in range(HL)
                    ]

                    def emit_scores(kc, q0=q0, b0=b0, b=b):
                        kcr = kc // 4
                        pump_until(("k", b, kcr))
                        st = ps_st.tile([128, 2 * QW], F32, tag="st", name="st")
                        for h in range(HL):
                            hs = slice(h * D, (h + 1) * D)
                            nc.tensor.matmul(
                                st[:, h * QW : (h + 1) * QW],
                                k_sb[hs, b0 + kc * 128 : b0 + (kc + 1) * 128],
                                q_sb[hs, q0 : q0 + QW],
                                start=True,
                                stop=True,
                                tile_position=(h * D, 0),
                            )
                        return st

                    pend = emit_scores(0)
                    for kc in range(KCN):
                        nxt = emit_scores(kc + 1) if kc + 1 < KCN else None
                        if kc < KCN - 2:
                            pump(2 if len(bg) > 20 else 1)
                        for h in range(HL):
                            pump_until(("vaug", b, h, kc // 4))
                        pt = ptp.tile([128, 2 * QW], MMDT, tag="pt")
                        nc.scalar.activation(
                            out=pt[:],
                            in_=pend[:],
                            func=mybir.ActivationFunctionType.Exp,
                            scale=SCALE,
                        )
                        for h in range(HL):
                            nc.tensor.matmul(
                                vps[h][:],
                                vaug_store[(b, h, kc // 4)][:, kc % 4, :],
                                pt[:, h * QW : (h + 1) * QW],
                                start=(kc == 0),
                                stop=(kc == KCN - 1),
                            )
                        pend = nxt

                    # drain vps to SBUF fast (frees PSUM for the next chunk)
                    qs = slice(qh * QW, (qh + 1) * QW)
                    for h in range(HL):
                        nc.vector.tensor_copy(out=outu[h][:, qs], in_=vps[h][:D, :])
                        nc.vector.tensor_copy(
                            out=rs[h][:, qs], in_=vps[h][D : D + 1, :]
                        )

                    # normalize off the critical path; queue this q-range's
                    # projection chunks as background work
                    def norm_and_proj(b0=b0, qh=qh, outt=outt, outu=outu, rs=rs, b=b):
                        for h in range(HL):
                            qs = slice(qh * QW, (qh + 1) * QW)
                            rb = rp.tile([D, QW], F32, tag="rb", name="rb")
                            nc.gpsimd.partition_broadcast(rb[:], rs[h][:, qs])
                            rbr = rp.tile([D, QW], F32, tag="rbr", name="rbr")
                            nc.vector.reciprocal_approx_fast(out=rbr[:], in_=rb[:])
                            nc.vector.tensor_mul(
                                out=outt[h * D : (h + 1) * D, qs],
                                in0=outu[h][:, qs],
                                in1=rbr[:],
                            )
                        tail = b == B - 1 and qh == NQH - 1
                        for s2 in range(qh * (QW // 128), (qh + 1) * (QW // 128)):
                            for nck in range(C // 512):
                                ua = tail and (s2 + nck) % 2 == 1
                                bg.append(
                                    (
                                        None,
                                        lambda outt=outt, b0=b0, s2=s2, nck=nck, ua=ua: (
                                            emit_proj_chunk(outt, b0, s2, nck, ua)
                                        ),
                                    )
                                )

                    bg.append((None, norm_and_proj))

            # drain remaining background work
            while bg:
                pump(1)
    nc.compile()
    return nc


_NC_CACHE = {}


def _get_nc():
    if "nc" not in _NC_CACHE:
        _NC_CACHE["nc"] = build_nc()
    return _NC_CACHE["nc"]


def make_in_maps(x, w_qkv, w_proj):
    np_dt = mybir.dt.np(MMDT)
    x = np.asarray(x, dtype=np.float32)
    w_qkv = np.asarray(w_qkv, dtype=np.float32)
    w_proj = np.asarray(w_proj, dtype=np.float32)
    xt = np.ascontiguousarray(x.reshape(SEQ, C).T.astype(np_dt))
    in_maps = []
    for c in range(NCORES):
        cs = slice(128 * c, 128 * c + 128)
        wslice = np.ascontiguousarray(
            np.concatenate(
                [w_qkv[:, cs], w_qkv[:, C:][:, cs], w_qkv[:, 2 * C :][:, cs]], axis=1
            ).astype(np_dt)
        )
        in_maps.append(
            {
                "xt": xt,
                "wqkv": wslice,
                "wproj": np.ascontiguousarray(w_proj[cs, :].astype(np_dt)),
            }
        )
    return in_maps


def kernel(x, w_qkv, w_proj, b_proj, _run_kwargs=None):
    nc = _get_nc()
    in_maps = make_in_maps(x, w_qkv, w_proj)
    res = run_bass_kernel_spmd(
        nc, in_maps, core_ids=list(range(NCORES)), **(_run_kwargs or {})
    )
    acc = res.results[0]["out"].astype(np.float32)
    for c in range(1, NCORES):
        acc = acc + res.results[c]["out"]
    acc = acc + np.asarray(b_proj, dtype=np.float32)[None, :]
    out = acc.reshape(B, N, C)
    if _run_kwargs:
        kernel.last_result = res
    return out
